# revision 36
# baseline (speedup 1.0000x reference)
"""Trainium2 Bass kernel for nn_AttentionHeadless (sparse_attention).

Reference computation (B=2, Q=512, K=512, T=256):
    k = key @ Wk.T; q = query @ Wq.T; v = value @ Wva.T
    logits[b,kk,q,u] = sum_t Wal[u,t] * k[b,kk,t] * q[b,q,t]
    scale = swishmax(logits, axis=-2)      # normalize over Q
    out = (sum_kk v[b,kk] * scale) @ Wvo.T

Sharding: data-parallel over (b, kk): each of 8 cores takes 64 of the 512
K-rows per batch. The q/k/v projections run on the host (0.2% of FLOPs),
as do the final Wvo matmul and the 8-way partial sum — both commute with
the per-core value-sum, so each core emits a partial [B, T, Q] output.

Per-core pipeline, layout [u on 128 partitions (2 chunks uc), q free].
With y = L*exp(L-M) and E = exp(L-M), the swishmax denominator is
    den = sum_q|y| + max_q E          (exactly, for any shift M)
so no max-recovery/Newton pass is needed: max_q E comes from a 4x-mode
tensor_scalar max-accumulate over E, and sum_q|y| from relu identities:
    uc0: sum|y| = 2*sum(relu(y)) - sum(y)   (sum(y) free from the fused
         multiply's add-accumulator)
    uc1: sum|y| = sum(relu(y)) - sum(min(y,0)), or a single ACT Abs with
         add-accumulate for 1-in-4 rows (load balancing)
Engine split per k-row (cost-model ns; DVE/ACT are the ~98%-busy pair):
    PE   main matmul fp16 (853) + diag-accumulate fp16 (427)
    ACT  E = Exp(lps) [both uc] (1038) + Lc = Copy(lps-uc1) fp16 (612)
         + 1-in-4 Abs-accum (~200)
    DVE  walk-uc0 (127), y0 = lps*E custom mul w/ sum-accum (658, the
         only f32-PSUM read), relu+(y0), maxE x2 (3 x 194), 3-in-4 relu
         pair on y1 (~291), diag builds (186), smalls (~47)
    GPS  y1 = Lc*E1 tensor_mul (1111+95) + walk-uc1 broadcast mul (638)
GPSIMD cannot touch PSUM (hence the ACT fp16 copy of the uc1 logits) and
runs only TensorTensor-class ops (no tensor_scalar / activations).

Schedule: one flat skewed software pipeline over all 128 (b, kk) rows —
every cross-engine consumer reads data >=1 row old (walk made 2 rows
ahead; Pool multiplies row g-1; reductions read rows g-1/g-2; the value
accumulation consumes batch G during batch G+2), so the in-order engines
never head-of-line block on fresh output. PSUM: 3-deep logits ring (12KB)
+ one [P,2,Q] f32 accumulator (4KB).
"""

import numpy as np

import concourse.bacc as bacc
import concourse.mybir as mybir
import concourse.tile as tile
from concourse import dve_ops
from concourse.bass_utils import run_bass_kernel_spmd
from concourse.dve_spec import Spec, Src0, Src1, AluOp, lower as _uop_lower
from concourse.dve_uop import DveOpSpec

B, Q, K, T = 2, 512, 512, 256
NCORES = 8
KSH = K // NCORES  # 64 K-rows per core per batch
BATCH = 16
HB = 8
MSHIFT = 3.0
P = 128

f32 = mybir.dt.float32
f32r = mybir.dt.float32r
fp16 = mybir.dt.float16
AF = mybir.ActivationFunctionType
ALU = mybir.AluOpType


def _register_dve_op(name, spec, subdim=False):
    for op in dve_ops.OPS:
        if op.name == name:
            return op
    shas = {}
    for ver in ("v3", "v4"):
        try:
            uops = _uop_lower(spec, ver=ver)
            shas[ver] = DveOpSpec(name=name, uops=uops).sha(ver)
        except Exception:
            pass
    op = dve_ops.DveOp(name, spec, subdim=subdim, uops_sha=shas)
    dve_ops.OPS.append(op)
    dve_ops._SUB_OPCODE_FOR_NAME[name] = (
        dve_ops._CUSTOM_DVE_ROW_BASE + len(dve_ops.OPS) - 1
    )
    dve_ops.CUSTOM_DVE_SPECS[name] = spec
    return op


def _ref_mul_addacc(in0, in1, c0, c1, c2):
    b = (in0.astype(np.float32) * in1.astype(np.float32)).astype(np.float32)
    return b, b.reshape(b.shape[0], -1).sum(axis=-1, keepdims=True)


MUL_ADDACC = _register_dve_op(
    "MUL_ADDACC_ANT",
    Spec(body=Src0 * Src1, accum=AluOp.ADD, reference=_ref_mul_addacc),
)

# kept for compatibility with older helper scripts
def _ref_mul_maxacc(in0, in1, c0, c1, c2):
    b = (in0.astype(np.float32) * in1.astype(np.float32)).astype(np.float32)
    return b, b.reshape(b.shape[0], -1).max(axis=-1, keepdims=True)


MUL_MAXACC = _register_dve_op(
    "MUL_MAXACC_ANT",
    Spec(body=Src0 * Src1, accum=AluOp.MAX, reference=_ref_mul_maxacc),
)


def build(n_cores=NCORES):
    nc = bacc.Bacc("TRN2", target_bir_lowering=False, debug=False, num_devices=n_cores)

    # ---- DRAM I/O (per-core); q/k/v projections are applied on the host ----
    d_walT = nc.dram_tensor("walT", [T, T], fp16, kind="ExternalInput").ap()
    d_qpT = nc.dram_tensor("qpT", [B, T, Q], fp16, kind="ExternalInput").ap()
    d_kp = nc.dram_tensor("kp", [B, T, KSH], f32, kind="ExternalInput").ap()
    d_vp = nc.dram_tensor("vp", [B, T, KSH], f32, kind="ExternalInput").ap()
    d_eye = nc.dram_tensor("eye", [P, P], fp16, kind="ExternalInput").ap()
    d_out = nc.dram_tensor("outT", [B, T, Q], f32, kind="ExternalOutput").ap()

    NB = KSH // BATCH  # batches per b

    with tile.TileContext(nc) as tc:
        cpool = tc.alloc_tile_pool(name="consts", bufs=1)
        lps_pool = tc.alloc_tile_pool(name="lps", bufs=3, space="PSUM")
        acc_pool = tc.alloc_tile_pool(name="accp", bufs=1, space="PSUM")
        walk_pool = tc.alloc_tile_pool(name="walk", bufs=10)
        e_pool = tc.alloc_tile_pool(name="epool", bufs=7)
        lc_pool = tc.alloc_tile_pool(name="lcpool", bufs=7)
        y_pool = tc.alloc_tile_pool(name="ypool", bufs=3)
        red_pool = tc.alloc_tile_pool(name="red", bufs=8)
        sm_pool = tc.alloc_tile_pool(name="smalls", bufs=3)
        diag_pool = tc.alloc_tile_pool(name="diag", bufs=12)
        scrap_pool = tc.alloc_tile_pool(name="scrap", bufs=16)
        out_pool = tc.alloc_tile_pool(name="outp", bufs=2)

        # ---- load constants (projections already applied on host) ----
        walT = cpool.tile([P, 2, T], fp16, tag="walT")
        eye = cpool.tile([P, P], fp16, tag="eye")
        qpT = cpool.tile([P, B, 2, Q], fp16, tag="qpT")
        kp = cpool.tile([P, B, 2, KSH], f32, tag="kp")
        vp = cpool.tile([P, B, 2, KSH], f32, tag="vp")
        # load order: everything the first pipeline steps need comes first
        nc.sync.dma_start(
            kp[:, :, :, :], d_kp.rearrange("b (sc p) k -> p b sc k", p=P)
        )
        for sc in range(2):
            nc.sync.dma_start(walT[:, sc, :], d_walT[sc * P : (sc + 1) * P, :])
        nc.sync.dma_start(
            qpT[:, 0, :, :], d_qpT.rearrange("b (sc p) q -> p b sc q", p=P)[:, 0]
        )
        nc.sync.dma_start(eye[:], d_eye)
        nc.sync.dma_start(
            qpT[:, 1, :, :], d_qpT.rearrange("b (sc p) q -> p b sc q", p=P)[:, 1]
        )
        nc.sync.dma_start(
            vp[:, :, :, :], d_vp.rearrange("b (sc p) k -> p b sc k", p=P)
        )

        biasM = cpool.tile([P, 1], f32, tag="biasM")
        nc.vector.memset(biasM[:], -MSHIFT)

        # ---- main loop: one flat skewed stream over all (b, kk) rows ----
        # Every cross-engine consumer reads data produced >=1 row earlier, so
        # the in-order engines never head-of-line block on fresh output:
        #   walk(g) made two rows ahead of its PE matmul
        #   Pool multiplies row g-1 (ACT outputs a full row old)
        #   uc0/maxE reductions read row g-1; uc1 reductions row g-2
        #   value-accumulation consumes global batch G during batch G+2, so
        #   b0's drain overlaps b1's ramp and the pipeline never empties
        ACCD = 2
        NG = B * KSH           # 128 global rows
        NGB = NG // BATCH      # 8 global batches
        NBB = KSH // BATCH     # 4 batches per b
        bst = {}
        acc_tiles = {}

        def get_acc(b):
            if b not in acc_tiles:
                acc_tiles[b] = acc_pool.tile(
                    [P, 2, Q], f32, tag="acc", name=f"acc_{b}"
                )
            return acc_tiles[b]

        def get_bst(G):
            if G not in bst:
                bst[G] = dict(
                    yring=y_pool.tile([P, BATCH, 2, Q], fp16, tag="yring", name=f"yring_{G}"),
                    sy0=red_pool.tile([P, BATCH], f32, tag="sy0", name=f"sy0_{G}"),
                    sp=red_pool.tile([P, 2, BATCH], f32, tag="sp", name=f"sp_{G}"),
                    sm1=red_pool.tile([P, BATCH], f32, tag="sm1", name=f"sm1_{G}"),
                    me=red_pool.tile([P, 2, BATCH], f32, tag="me", name=f"me_{G}"),
                    den=sm_pool.tile([P, 2, BATCH], f32, tag="den", name=f"den_{G}"),
                    cc=sm_pool.tile([P, 2, BATCH], f32, tag="cc", name=f"cc_{G}"),
                )
                nc.gpsimd.memset(bst[G]["sm1"][:], 0.0)
            return bst[G]

        walk_tiles, E_tiles, Lc_tiles, lps_tiles = {}, {}, {}, {}

        def emit_walk(g):
            b, kk = g // KSH, g % KSH
            w = walk_pool.tile([P, 2, T], fp16, tag="walk")
            nc.vector.tensor_scalar_mul(
                w[:, 0, :], walT[:, 0, :], kp[:, b, 0, kk : kk + 1]
            )
            nc.gpsimd.tensor_mul(
                w[:, 1, :], walT[:, 1, :],
                kp[:, b, 1, kk : kk + 1].broadcast_to([P, T]),
            )
            walk_tiles[g] = w

        def emit_mm_exp(g):
            b = g // KSH
            walk = walk_tiles.pop(g)
            lps = lps_pool.tile([P, 2, Q], f32, tag="lps")
            for uc in range(2):
                for t_c in range(2):
                    nc.tensor.matmul(
                        lps[:, uc, :],
                        walk[:, t_c, uc * P : (uc + 1) * P],
                        qpT[:, b, t_c, :],
                        start=(t_c == 0),
                        stop=(t_c == 1),
                    )
            E = e_pool.tile([P, 2, Q], fp16, tag="E")
            nc.scalar.activation(
                E[:, :, :], lps[:, :, :], AF.Exp, bias=biasM[:], scale=1.0
            )
            Lc = lc_pool.tile([P, Q], fp16, tag="Lc")
            nc.scalar.activation(Lc[:], lps[:, 1, :], AF.Copy, bias=0.0, scale=1.0)
            E_tiles[g], Lc_tiles[g], lps_tiles[g] = E, Lc, lps

        def emit_y0(g):  # DVE fused multiply, same-step (reads lps+E)
            st = get_bst(g // BATCH)
            j = g % BATCH
            nc.vector._custom_dve(
                MUL_ADDACC,
                out=st["yring"][:, j, 0, :],
                in0=lps_tiles.pop(g)[:, 0, :],
                in1=E_tiles[g][:, 0, :],
                accum_out=st["sy0"][:, j : j + 1],
            )

        def emit_y1(g):  # Pool multiply, one row behind
            st = get_bst(g // BATCH)
            nc.gpsimd.tensor_mul(
                st["yring"][:, g % BATCH, 1, :],
                Lc_tiles.pop(g)[:],
                E_tiles[g][:, 1, :],
            )

        def emit_reds0(g):  # relu+(y0) and maxE both uc, row g-1
            st = get_bst(g // BATCH)
            j = g % BATCH
            scr = scrap_pool.tile([P, Q], fp16, tag="scr")
            nc.vector.tensor_scalar(
                scr[:], st["yring"][:, j, 0, :], 0.0, None,
                op0=ALU.max, op1=ALU.add,
                accum_out=st["sp"][:, 0, j : j + 1],
            )
            E = E_tiles.pop(g)
            for uc in range(2):
                scrE = scrap_pool.tile([P, Q], fp16, tag="scr")
                nc.vector.tensor_scalar(
                    scrE[:], E[:, uc, :], 1.0, None,
                    op0=ALU.mult, op1=ALU.max,
                    accum_out=st["me"][:, uc, j : j + 1],
                )

        def emit_reds1(g):  # sum|y1|: ACT Abs for 1-in-4 rows, else relu pair
            st = get_bst(g // BATCH)
            j = g % BATCH
            if g % 4 == 0 or g in (30, 62, 94):
                # one ACT pass: sp1 = sum|y1|; sm1 stays 0 (memset)
                scr = scrap_pool.tile([P, Q], fp16, tag="scr")
                nc.scalar.activation(
                    scr[:], st["yring"][:, j, 1, :], AF.Abs,
                    accum_out=st["sp"][:, 1, j : j + 1],
                )
                return
            scr = scrap_pool.tile([P, Q], fp16, tag="scr")
            nc.vector.tensor_scalar(
                scr[:], st["yring"][:, j, 1, :], 0.0, None,
                op0=ALU.max, op1=ALU.add,
                accum_out=st["sp"][:, 1, j : j + 1],
            )
            scrm = scrap_pool.tile([P, Q], fp16, tag="scr")
            nc.vector.tensor_scalar(
                scrm[:], st["yring"][:, j, 1, :], 0.0, None,
                op0=ALU.min, op1=ALU.add,
                accum_out=st["sm1"][:, j : j + 1],
            )

        def emit_smalls(G, h):
            st = get_bst(G)
            b, kk0 = (G * BATCH) // KSH, (G * BATCH) % KSH
            sl = slice(h * HB, (h + 1) * HB)
            den, cc, sp, sy0, sm1, me = (
                st["den"], st["cc"], st["sp"], st["sy0"], st["sm1"], st["me"]
            )
            nc.vector.scalar_tensor_tensor(
                den[:, 0, sl], sp[:, 0, sl], 2.0, sy0[:, sl],
                op0=ALU.mult, op1=ALU.subtract,
            )
            nc.vector.tensor_sub(den[:, 1, sl], sp[:, 1, sl], sm1[:, sl])
            nc.vector.tensor_add(den[:, :, sl], den[:, :, sl], me[:, :, sl])
            nc.vector.reciprocal_approx_fast(den[:, :, sl], den[:, :, sl])
            nc.vector.tensor_mul(
                cc[:, :, sl], den[:, :, sl],
                vp[:, b, :, kk0 + h * HB : kk0 + (h + 1) * HB],
            )

        def acc_pair(G, j):
            st = bst[G]
            b = (G * BATCH) // KSH
            acc = get_acc(b)
            for uc in range(2):
                diagt = diag_pool.tile([P, P], fp16, tag="diagt")
                nc.vector.tensor_scalar_mul(
                    diagt[:], eye[:], st["cc"][:, uc, j : j + 1]
                )
                nc.tensor.matmul(
                    acc[:, uc, :],
                    diagt[:],
                    st["yring"][:, j, uc, :],
                    start=(G % NBB == 0 and j == 0),
                    stop=(G % NBB == NBB - 1 and j == BATCH - 1),
                    skip_group_check=True,
                )
            if G % NBB == NBB - 1 and j == BATCH - 1:
                # this b's accumulation group just closed: drain it
                st_out = out_pool.tile([P, 2, Q], f32, tag="st")
                nc.scalar.copy(st_out[:, :, :], acc[:, :, :])
                for sc in range(2):
                    nc.sync.dma_start(
                        d_out[b, sc * P : (sc + 1) * P, :], st_out[:, sc, :]
                    )
                del acc_tiles[b]

        smalls_done = set()

        def drain_smalls(g):
            # half (G, h) ready once reds1 of its last row (G*16+h*8+7)
            # has been emitted, i.e. at step >= that row + 2
            for G in range(NGB):
                for h in range(2):
                    key = (G, h)
                    if key in smalls_done:
                        continue
                    if g >= G * BATCH + h * HB + HB - 1 + 2:
                        emit_smalls(G, h)
                        smalls_done.add(key)

        emit_walk(0)
        emit_walk(1)
        for g in range(NG):
            G, j = g // BATCH, g % BATCH
            if G >= ACCD:
                acc_pair(G - ACCD, j)
            if g + 2 < NG:
                emit_walk(g + 2)
            emit_mm_exp(g)
            if g >= 1:
                emit_y1(g - 1)
                emit_reds0(g - 1)
            if g >= 2:
                emit_reds1(g - 2)
            emit_y0(g)
            drain_smalls(g)

        # epilogue: trailing rows, smalls, last two value-acc batches + drain
        emit_y1(NG - 1)
        emit_reds0(NG - 1)
        emit_reds1(NG - 2)
        emit_reds1(NG - 1)
        drain_smalls(NG + 1)
        for G in range(NGB - ACCD, NGB):
            for j in range(BATCH):
                acc_pair(G, j)

        for pl in (out_pool, scrap_pool, diag_pool, sm_pool, red_pool, y_pool,
                   lc_pool, e_pool, walk_pool, acc_pool, lps_pool, cpool):
            pl.release()

    nc.compile()
    return nc


_NC_CACHE = {}


def _get_nc(n_cores=NCORES):
    if n_cores not in _NC_CACHE:
        _NC_CACHE[n_cores] = build(n_cores)
    return _NC_CACHE[n_cores]


def make_in_maps(query_tokens, key_tokens, value_tokens, Wk, Wq, Wva, Wal, Wvo):
    # host-side projections (tiny vs the on-device [B,K,Q,T] work)
    qp = query_tokens.astype(np.float32) @ Wq.T.astype(np.float32)   # [B,Q,T]
    kpj = key_tokens.astype(np.float32) @ Wk.T.astype(np.float32)    # [B,K,T]
    vpj = value_tokens.astype(np.float32) @ Wva.T.astype(np.float32)
    qpT = np.ascontiguousarray(np.transpose(qp, (0, 2, 1))).astype(np.float16)
    kpT = np.ascontiguousarray(np.transpose(kpj, (0, 2, 1)), np.float32)
    vpT = np.ascontiguousarray(np.transpose(vpj, (0, 2, 1)), np.float32)
    walT = np.ascontiguousarray(Wal.T).astype(np.float16)
    eye = np.eye(P, dtype=np.float16)
    in_maps = []
    for c in range(NCORES):
        sl = slice(c * KSH, (c + 1) * KSH)
        in_maps.append(
            {
                "walT": walT,
                "qpT": qpT,
                "kp": np.ascontiguousarray(kpT[:, :, sl]),
                "vp": np.ascontiguousarray(vpT[:, :, sl]),
                "eye": eye,
            }
        )
    return in_maps


def kernel(query_tokens, key_tokens, value_tokens, Wk, Wq, Wva, Wal, Wvo):
    args = [np.asarray(a, np.float32) for a in
            (query_tokens, key_tokens, value_tokens, Wk, Wq, Wva, Wal, Wvo)]
    in_maps = make_in_maps(*args)
    nc = _get_nc()
    res = run_bass_kernel_spmd(nc, in_maps, core_ids=list(range(NCORES)))
    total = np.zeros((B, T, Q), np.float32)
    for c in range(NCORES):
        total += res.results[c]["outT"]
    # total is the value-sum transposed [B, T, Q]; apply Wvo on host
    Wvo = np.asarray(args[7], np.float32)
    return np.einsum("ut,btq->bqu", Wvo, total).astype(np.float32)


# revision 37
# speedup vs baseline: 1.0056x; 1.0056x over previous
"""Trainium2 Bass kernel for nn_AttentionHeadless (sparse_attention).

Reference computation (B=2, Q=512, K=512, T=256):
    k = key @ Wk.T; q = query @ Wq.T; v = value @ Wva.T
    logits[b,kk,q,u] = sum_t Wal[u,t] * k[b,kk,t] * q[b,q,t]
    scale = swishmax(logits, axis=-2)      # normalize over Q
    out = (sum_kk v[b,kk] * scale) @ Wvo.T

Sharding: data-parallel over (b, kk): each of 8 cores takes 64 of the 512
K-rows per batch. The q/k/v projections run on the host (0.2% of FLOPs),
as do the final Wvo matmul and the 8-way partial sum — both commute with
the per-core value-sum, so each core emits a partial [B, T, Q] output.

Per-core pipeline, layout [u on 128 partitions (2 chunks uc), q free].
With y = L*exp(L-M) and E = exp(L-M), the swishmax denominator is
    den = sum_q|y| + max_q E          (exactly, for any shift M)
so no max-recovery/Newton pass is needed: max_q E comes from a 4x-mode
tensor_scalar max-accumulate over E, and sum_q|y| from relu identities:
    uc0: sum|y| = 2*sum(relu(y)) - sum(y)   (sum(y) free from the fused
         multiply's add-accumulator)
    uc1: sum|y| = sum(relu(y)) - sum(min(y,0)), or a single ACT Abs with
         add-accumulate for 1-in-4 rows (load balancing)
Engine split per k-row (cost-model ns; DVE/ACT are the ~98%-busy pair):
    PE   main matmul fp16 (853) + diag-accumulate fp16 (427)
    ACT  E = Exp(lps) [both uc] (1038) + Lc = Copy(lps-uc1) fp16 (612)
         + 1-in-4 Abs-accum (~200)
    DVE  walk-uc0 (127), y0 = lps*E custom mul w/ sum-accum (658, the
         only f32-PSUM read), relu+(y0), maxE x2 (3 x 194), 3-in-4 relu
         pair on y1 (~291), diag builds (186), smalls (~47)
    GPS  y1 = Lc*E1 tensor_mul (1111+95) + walk-uc1 broadcast mul (638)
GPSIMD cannot touch PSUM (hence the ACT fp16 copy of the uc1 logits) and
runs only TensorTensor-class ops (no tensor_scalar / activations).

Schedule: one flat skewed software pipeline over all 128 (b, kk) rows —
every cross-engine consumer reads data >=1 row old (walk made 2 rows
ahead; Pool multiplies row g-1; reductions read rows g-1/g-2; the value
accumulation consumes batch G during batch G+2), so the in-order engines
never head-of-line block on fresh output. PSUM: 3-deep logits ring (12KB)
+ one [P,2,Q] f32 accumulator (4KB).
"""

import numpy as np

import concourse.bacc as bacc
import concourse.mybir as mybir
import concourse.tile as tile
from concourse import dve_ops
from concourse.bass_utils import run_bass_kernel_spmd
from concourse.dve_spec import Spec, Src0, Src1, AluOp, lower as _uop_lower
from concourse.dve_uop import DveOpSpec

B, Q, K, T = 2, 512, 512, 256
NCORES = 8
KSH = K // NCORES  # 64 K-rows per core per batch
BATCH = 16
HB = 8
MSHIFT = 3.0
P = 128

f32 = mybir.dt.float32
f32r = mybir.dt.float32r
fp16 = mybir.dt.float16
AF = mybir.ActivationFunctionType
ALU = mybir.AluOpType


def _register_dve_op(name, spec, subdim=False):
    for op in dve_ops.OPS:
        if op.name == name:
            return op
    shas = {}
    for ver in ("v3", "v4"):
        try:
            uops = _uop_lower(spec, ver=ver)
            shas[ver] = DveOpSpec(name=name, uops=uops).sha(ver)
        except Exception:
            pass
    op = dve_ops.DveOp(name, spec, subdim=subdim, uops_sha=shas)
    dve_ops.OPS.append(op)
    dve_ops._SUB_OPCODE_FOR_NAME[name] = (
        dve_ops._CUSTOM_DVE_ROW_BASE + len(dve_ops.OPS) - 1
    )
    dve_ops.CUSTOM_DVE_SPECS[name] = spec
    return op


def _ref_mul_addacc(in0, in1, c0, c1, c2):
    b = (in0.astype(np.float32) * in1.astype(np.float32)).astype(np.float32)
    return b, b.reshape(b.shape[0], -1).sum(axis=-1, keepdims=True)


MUL_ADDACC = _register_dve_op(
    "MUL_ADDACC_ANT",
    Spec(body=Src0 * Src1, accum=AluOp.ADD, reference=_ref_mul_addacc),
)

# kept for compatibility with older helper scripts
def _ref_mul_maxacc(in0, in1, c0, c1, c2):
    b = (in0.astype(np.float32) * in1.astype(np.float32)).astype(np.float32)
    return b, b.reshape(b.shape[0], -1).max(axis=-1, keepdims=True)


MUL_MAXACC = _register_dve_op(
    "MUL_MAXACC_ANT",
    Spec(body=Src0 * Src1, accum=AluOp.MAX, reference=_ref_mul_maxacc),
)


def build(n_cores=NCORES):
    nc = bacc.Bacc("TRN2", target_bir_lowering=False, debug=False, num_devices=n_cores)

    # ---- DRAM I/O (per-core); q/k/v projections are applied on the host ----
    d_walT = nc.dram_tensor("walT", [T, T], fp16, kind="ExternalInput").ap()
    d_qpT = nc.dram_tensor("qpT", [B, T, Q], fp16, kind="ExternalInput").ap()
    d_kp = nc.dram_tensor("kp", [B, T, KSH], f32, kind="ExternalInput").ap()
    d_vp = nc.dram_tensor("vp", [B, T, KSH], f32, kind="ExternalInput").ap()
    d_eye = nc.dram_tensor("eye", [P, P], fp16, kind="ExternalInput").ap()
    d_out = nc.dram_tensor("outT", [B, T, Q], f32, kind="ExternalOutput").ap()

    NB = KSH // BATCH  # batches per b

    with tile.TileContext(nc) as tc:
        cpool = tc.alloc_tile_pool(name="consts", bufs=1)
        lps_pool = tc.alloc_tile_pool(name="lps", bufs=3, space="PSUM")
        acc_pool = tc.alloc_tile_pool(name="accp", bufs=1, space="PSUM")
        walk_pool = tc.alloc_tile_pool(name="walk", bufs=10)
        e_pool = tc.alloc_tile_pool(name="epool", bufs=7)
        lc_pool = tc.alloc_tile_pool(name="lcpool", bufs=7)
        y_pool = tc.alloc_tile_pool(name="ypool", bufs=3)
        red_pool = tc.alloc_tile_pool(name="red", bufs=8)
        sm_pool = tc.alloc_tile_pool(name="smalls", bufs=3)
        diag_pool = tc.alloc_tile_pool(name="diag", bufs=12)
        scrap_pool = tc.alloc_tile_pool(name="scrap", bufs=16)
        out_pool = tc.alloc_tile_pool(name="outp", bufs=2)

        # ---- load constants (projections already applied on host) ----
        walT = cpool.tile([P, 2, T], fp16, tag="walT")
        eye = cpool.tile([P, P], fp16, tag="eye")
        qpT = cpool.tile([P, B, 2, Q], fp16, tag="qpT")
        kp = cpool.tile([P, B, 2, KSH], f32, tag="kp")
        vp = cpool.tile([P, B, 2, KSH], f32, tag="vp")
        # load order: everything the first pipeline steps need comes first
        nc.sync.dma_start(
            kp[:, :, :, :], d_kp.rearrange("b (sc p) k -> p b sc k", p=P)
        )
        for sc in range(2):
            nc.sync.dma_start(walT[:, sc, :], d_walT[sc * P : (sc + 1) * P, :])
        nc.sync.dma_start(
            qpT[:, 0, :, :], d_qpT.rearrange("b (sc p) q -> p b sc q", p=P)[:, 0]
        )
        nc.sync.dma_start(eye[:], d_eye)
        nc.sync.dma_start(
            qpT[:, 1, :, :], d_qpT.rearrange("b (sc p) q -> p b sc q", p=P)[:, 1]
        )
        nc.sync.dma_start(
            vp[:, :, :, :], d_vp.rearrange("b (sc p) k -> p b sc k", p=P)
        )

        biasM = cpool.tile([P, 1], f32, tag="biasM")
        nc.vector.memset(biasM[:], -MSHIFT)

        # ---- main loop: one flat skewed stream over all (b, kk) rows ----
        # Every cross-engine consumer reads data produced >=1 row earlier, so
        # the in-order engines never head-of-line block on fresh output:
        #   walk(g) made two rows ahead of its PE matmul
        #   Pool multiplies row g-1 (ACT outputs a full row old)
        #   uc0/maxE reductions read row g-1; uc1 reductions row g-2
        #   value-accumulation consumes global batch G during batch G+2, so
        #   b0's drain overlaps b1's ramp and the pipeline never empties
        ACCD = 2
        NG = B * KSH           # 128 global rows
        NGB = NG // BATCH      # 8 global batches
        NBB = KSH // BATCH     # 4 batches per b
        bst = {}
        acc_tiles = {}

        def get_acc(b):
            if b not in acc_tiles:
                acc_tiles[b] = acc_pool.tile(
                    [P, 2, Q], f32, tag="acc", name=f"acc_{b}"
                )
            return acc_tiles[b]

        def get_bst(G):
            if G not in bst:
                bst[G] = dict(
                    yring=y_pool.tile([P, BATCH, 2, Q], fp16, tag="yring", name=f"yring_{G}"),
                    sy0=red_pool.tile([P, BATCH], f32, tag="sy0", name=f"sy0_{G}"),
                    sp=red_pool.tile([P, 2, BATCH], f32, tag="sp", name=f"sp_{G}"),
                    sm1=red_pool.tile([P, BATCH], f32, tag="sm1", name=f"sm1_{G}"),
                    me=red_pool.tile([P, 2, BATCH], f32, tag="me", name=f"me_{G}"),
                    den=sm_pool.tile([P, 2, BATCH], f32, tag="den", name=f"den_{G}"),
                    cc=sm_pool.tile([P, 2, BATCH], f32, tag="cc", name=f"cc_{G}"),
                )
                nc.vector.memset(bst[G]["sm1"][:], 0.0)
            return bst[G]

        walk_tiles, E_tiles, Lc_tiles, lps_tiles = {}, {}, {}, {}

        def emit_walk(g):
            b, kk = g // KSH, g % KSH
            w = walk_pool.tile([P, 2, T], fp16, tag="walk")
            nc.vector.tensor_scalar_mul(
                w[:, 0, :], walT[:, 0, :], kp[:, b, 0, kk : kk + 1]
            )
            nc.gpsimd.tensor_mul(
                w[:, 1, :], walT[:, 1, :],
                kp[:, b, 1, kk : kk + 1].broadcast_to([P, T]),
            )
            walk_tiles[g] = w

        def emit_mm_exp(g):
            b = g // KSH
            walk = walk_tiles.pop(g)
            lps = lps_pool.tile([P, 2, Q], f32, tag="lps")
            for uc in range(2):
                for t_c in range(2):
                    nc.tensor.matmul(
                        lps[:, uc, :],
                        walk[:, t_c, uc * P : (uc + 1) * P],
                        qpT[:, b, t_c, :],
                        start=(t_c == 0),
                        stop=(t_c == 1),
                    )
            E = e_pool.tile([P, 2, Q], fp16, tag="E")
            nc.scalar.activation(
                E[:, :, :], lps[:, :, :], AF.Exp, bias=biasM[:], scale=1.0
            )
            Lc = lc_pool.tile([P, Q], fp16, tag="Lc")
            nc.scalar.activation(Lc[:], lps[:, 1, :], AF.Copy, bias=0.0, scale=1.0)
            E_tiles[g], Lc_tiles[g], lps_tiles[g] = E, Lc, lps

        def emit_y0(g):  # DVE fused multiply, same-step (reads lps+E)
            st = get_bst(g // BATCH)
            j = g % BATCH
            nc.vector._custom_dve(
                MUL_ADDACC,
                out=st["yring"][:, j, 0, :],
                in0=lps_tiles.pop(g)[:, 0, :],
                in1=E_tiles[g][:, 0, :],
                accum_out=st["sy0"][:, j : j + 1],
            )

        def emit_y1(g):  # Pool multiply, one row behind
            st = get_bst(g // BATCH)
            nc.gpsimd.tensor_mul(
                st["yring"][:, g % BATCH, 1, :],
                Lc_tiles.pop(g)[:],
                E_tiles[g][:, 1, :],
            )

        def emit_reds0(g):  # relu+(y0) and maxE both uc, row g-1
            st = get_bst(g // BATCH)
            j = g % BATCH
            scr = scrap_pool.tile([P, Q], fp16, tag="scr")
            nc.vector.tensor_scalar(
                scr[:], st["yring"][:, j, 0, :], 0.0, None,
                op0=ALU.max, op1=ALU.add,
                accum_out=st["sp"][:, 0, j : j + 1],
            )
            E = E_tiles.pop(g)
            for uc in range(2):
                scrE = scrap_pool.tile([P, Q], fp16, tag="scr")
                nc.vector.tensor_scalar(
                    scrE[:], E[:, uc, :], 1.0, None,
                    op0=ALU.mult, op1=ALU.max,
                    accum_out=st["me"][:, uc, j : j + 1],
                )

        def emit_reds1(g):  # sum|y1|: ACT Abs for 1-in-4 rows, else relu pair
            st = get_bst(g // BATCH)
            j = g % BATCH
            if g % 4 == 0:
                # one ACT pass: sp1 = sum|y1|; sm1 stays 0 (memset)
                scr = scrap_pool.tile([P, Q], fp16, tag="scr")
                nc.scalar.activation(
                    scr[:], st["yring"][:, j, 1, :], AF.Abs,
                    accum_out=st["sp"][:, 1, j : j + 1],
                )
                return
            scr = scrap_pool.tile([P, Q], fp16, tag="scr")
            nc.vector.tensor_scalar(
                scr[:], st["yring"][:, j, 1, :], 0.0, None,
                op0=ALU.max, op1=ALU.add,
                accum_out=st["sp"][:, 1, j : j + 1],
            )
            scrm = scrap_pool.tile([P, Q], fp16, tag="scr")
            nc.vector.tensor_scalar(
                scrm[:], st["yring"][:, j, 1, :], 0.0, None,
                op0=ALU.min, op1=ALU.add,
                accum_out=st["sm1"][:, j : j + 1],
            )

        def emit_smalls(G, h):
            st = get_bst(G)
            b, kk0 = (G * BATCH) // KSH, (G * BATCH) % KSH
            sl = slice(h * HB, (h + 1) * HB)
            den, cc, sp, sy0, sm1, me = (
                st["den"], st["cc"], st["sp"], st["sy0"], st["sm1"], st["me"]
            )
            nc.vector.scalar_tensor_tensor(
                den[:, 0, sl], sp[:, 0, sl], 2.0, sy0[:, sl],
                op0=ALU.mult, op1=ALU.subtract,
            )
            nc.vector.tensor_sub(den[:, 1, sl], sp[:, 1, sl], sm1[:, sl])
            nc.vector.tensor_add(den[:, :, sl], den[:, :, sl], me[:, :, sl])
            nc.vector.reciprocal_approx_fast(den[:, :, sl], den[:, :, sl])
            nc.vector.tensor_mul(
                cc[:, :, sl], den[:, :, sl],
                vp[:, b, :, kk0 + h * HB : kk0 + (h + 1) * HB],
            )

        def acc_pair(G, j):
            st = bst[G]
            b = (G * BATCH) // KSH
            acc = get_acc(b)
            for uc in range(2):
                diagt = diag_pool.tile([P, P], fp16, tag="diagt")
                nc.vector.tensor_scalar_mul(
                    diagt[:], eye[:], st["cc"][:, uc, j : j + 1]
                )
                nc.tensor.matmul(
                    acc[:, uc, :],
                    diagt[:],
                    st["yring"][:, j, uc, :],
                    start=(G % NBB == 0 and j == 0),
                    stop=(G % NBB == NBB - 1 and j == BATCH - 1),
                    skip_group_check=True,
                )
            if G % NBB == NBB - 1 and j == BATCH - 1:
                # this b's accumulation group just closed: drain it
                st_out = out_pool.tile([P, 2, Q], f32, tag="st")
                nc.scalar.copy(st_out[:, :, :], acc[:, :, :])
                for sc in range(2):
                    nc.sync.dma_start(
                        d_out[b, sc * P : (sc + 1) * P, :], st_out[:, sc, :]
                    )
                del acc_tiles[b]

        smalls_done = set()

        def drain_smalls(g):
            # half (G, h) ready once reds1 of its last row (G*16+h*8+7)
            # has been emitted, i.e. at step >= that row + 2
            for G in range(NGB):
                for h in range(2):
                    key = (G, h)
                    if key in smalls_done:
                        continue
                    if g >= G * BATCH + h * HB + HB - 1 + 2:
                        emit_smalls(G, h)
                        smalls_done.add(key)

        emit_walk(0)
        emit_walk(1)
        for g in range(NG):
            G, j = g // BATCH, g % BATCH
            if G >= ACCD:
                acc_pair(G - ACCD, j)
            if g + 2 < NG:
                emit_walk(g + 2)
            emit_mm_exp(g)
            if g >= 1:
                emit_y1(g - 1)
                emit_reds0(g - 1)
            if g >= 2:
                emit_reds1(g - 2)
            emit_y0(g)
            drain_smalls(g)

        # epilogue: trailing rows, smalls, last two value-acc batches + drain
        emit_y1(NG - 1)
        emit_reds0(NG - 1)
        emit_reds1(NG - 2)
        emit_reds1(NG - 1)
        drain_smalls(NG + 1)
        for G in range(NGB - ACCD, NGB):
            for j in range(BATCH):
                acc_pair(G, j)

        for pl in (out_pool, scrap_pool, diag_pool, sm_pool, red_pool, y_pool,
                   lc_pool, e_pool, walk_pool, acc_pool, lps_pool, cpool):
            pl.release()

    nc.compile()
    return nc


_NC_CACHE = {}


def _get_nc(n_cores=NCORES):
    if n_cores not in _NC_CACHE:
        _NC_CACHE[n_cores] = build(n_cores)
    return _NC_CACHE[n_cores]


def make_in_maps(query_tokens, key_tokens, value_tokens, Wk, Wq, Wva, Wal, Wvo):
    # host-side projections (tiny vs the on-device [B,K,Q,T] work)
    qp = query_tokens.astype(np.float32) @ Wq.T.astype(np.float32)   # [B,Q,T]
    kpj = key_tokens.astype(np.float32) @ Wk.T.astype(np.float32)    # [B,K,T]
    vpj = value_tokens.astype(np.float32) @ Wva.T.astype(np.float32)
    qpT = np.ascontiguousarray(np.transpose(qp, (0, 2, 1))).astype(np.float16)
    kpT = np.ascontiguousarray(np.transpose(kpj, (0, 2, 1)), np.float32)
    vpT = np.ascontiguousarray(np.transpose(vpj, (0, 2, 1)), np.float32)
    walT = np.ascontiguousarray(Wal.T).astype(np.float16)
    eye = np.eye(P, dtype=np.float16)
    in_maps = []
    for c in range(NCORES):
        sl = slice(c * KSH, (c + 1) * KSH)
        in_maps.append(
            {
                "walT": walT,
                "qpT": qpT,
                "kp": np.ascontiguousarray(kpT[:, :, sl]),
                "vp": np.ascontiguousarray(vpT[:, :, sl]),
                "eye": eye,
            }
        )
    return in_maps


def kernel(query_tokens, key_tokens, value_tokens, Wk, Wq, Wva, Wal, Wvo):
    args = [np.asarray(a, np.float32) for a in
            (query_tokens, key_tokens, value_tokens, Wk, Wq, Wva, Wal, Wvo)]
    in_maps = make_in_maps(*args)
    nc = _get_nc()
    res = run_bass_kernel_spmd(nc, in_maps, core_ids=list(range(NCORES)))
    total = np.zeros((B, T, Q), np.float32)
    for c in range(NCORES):
        total += res.results[c]["outT"]
    # total is the value-sum transposed [B, T, Q]; apply Wvo on host
    Wvo = np.asarray(args[7], np.float32)
    return np.einsum("ut,btq->bqu", Wvo, total).astype(np.float32)
